# revision 19
# baseline (speedup 1.0000x reference)
"""Bicubic 4x downsample (MATLAB imresize-style) on Trainium2, 8-core data parallel.

Math: separable resize, H then W; both stages are banded matmuls evaluated
on the PE array with the image tile as the stationary operand:

  stage 1:  o1[w, oh]  = sum_h  X[h, w]  * WHT[h, oh]    (per 128x128 x-tile,
            streaming only the ~35-wide band window of WHT per h-tile)
  stage 2:  out[oh,ow] = sum_w  o1[w,oh] * WWT[w, ow]

Perf structure (single-shot exec time is the metric):
- x is cast to bf16 on the HOST: halves HBM read traffic (the kernel is
  HBM-stream-bound), and bf16 operands run the PE at 1 cycle/row.
- The output is stored bf16 (upcast on host); rel-err stays ~5e-3.
- All load DMAs ride ONE HWDGE queue (sync) in strict channel order; DMA
  trigger instructions are kept off the scalar/ACT engine, which runs the
  PSUM evictions (triggers are ~0.7us each and would delay evictions).
- s1 runs ht-major in two passes of (8, 7) w-tiles: channel 0's pass0
  tracks the load wave h-tile by h-tile instead of needing the whole
  channel resident; 8 PSUM banks are shared by the s1 accumulators and
  the 3 s2 accumulators via one rotating pool.
- o1 evictions alternate DVE/ACT so ps1 bank reuse never waits on one
  engine; output stores trigger on sync (idle after the load triggers).

Sharding: pure data parallel, batch b -> core b (8 batches, 8 cores).
"""

import numpy as np

TILE = 128


def _ensure_concourse():
    try:
        import concourse  # noqa: F401
    except ImportError:
        import sys
        for p in ("/opt/trn_rl_repo", "/root/.axon_site/_ro/trn_rl_repo"):
            if p not in sys.path:
                sys.path.insert(0, p)


_PATCHED = False


def _patch_tile_drain():
    """This walrus build rejects >1 sem wait on TPB_CTRL instructions (the
    Tile exit Drain). Split the final drain's waits into single-wait nops."""
    global _PATCHED
    if _PATCHED:
        return
    from concourse import tile
    from concourse.vector_clock import VectorClock, ScopedClock

    def _drain_and_barrier(self, tick_clock, wait_clock):
        gc = tick_clock.global_clock
        n = len(gc)
        for i in range(n):
            if gc[i] <= 0:
                continue
            vc = VectorClock([gc[j] if j == i else 0 for j in range(n)])
            nop_inst = self.nc.sync.nop(nofuse=True, hint="drain_split")
            wait_clock.add_sem_waits(nop_inst.ins, ScopedClock({None: vc}))
        self.nc.sync.drain()
        self.nc.all_engine_barrier()
        assert self.sems is not None
        popped = self.nc._tile_sem_poison_stack.pop()
        assert popped is self._sem_poison
        self.nc.clear_and_free_semaphores(list(self.sems.allocated().values()))
        self.nc.all_engine_barrier()

    tile.TileContext._drain_and_barrier = _drain_and_barrier
    _PATCHED = True


def _split_multi_waits(nc):
    """This walrus build rejects instructions carrying >1 sem wait. Hoist all
    but the last wait of any instruction onto same-engine nops placed
    immediately before it (engine streams execute block order in-order, so
    waiting on a preceding nop is equivalent)."""
    from concourse import mybir

    uid = 0
    for fn in nc.m.functions:
        for bb in fn.blocks:
            insts = bb.instructions  # live list
            new_list = []
            changed = False
            for ins in list(insts):
                si = ins.sync_info
                if si is not None and len(si.on_wait) > 1:
                    waits = list(si.on_wait)
                    for wt in waits[:-1]:
                        uid += 1
                        nop = mybir.InstNoOp(
                            name=f"ws_nop_{uid}",
                            engine=ins.engine,
                            ins=[],
                            outs=[],
                            sync_info=mybir.SyncInfo(on_wait=[wt], on_update=[]),
                            bass_nofuse=True,
                        )
                        new_list.append(nop)
                    ins.sync_info = mybir.SyncInfo(
                        on_wait=[waits[-1]], on_update=list(si.on_update)
                    )
                    changed = True
                new_list.append(ins)
            if changed:
                insts.clear()
                insts.extend(new_list)


def _dense_t(weights, indices, in_len):
    """Dense transposed resize matrix [in_len, out_len]:
    M[i, o] = sum over taps p with indices[o, p] == i of weights[o, p]."""
    w = np.asarray(weights, np.float32)
    idx = np.asarray(indices, np.int64)
    out_len, ntap = w.shape
    m = np.zeros((in_len, out_len), np.float32)
    ocol = np.repeat(np.arange(out_len), ntap)
    np.add.at(m, (idx.ravel(), ocol), w.ravel())
    return m


def _windows(mat_t):
    """Per 128-row tile of the [in, out] matrix: (out_lo, out_hi, packed_off)."""
    wins = []
    off = 0
    for t0 in range(0, mat_t.shape[0], TILE):
        blk = mat_t[t0:t0 + TILE]
        nz = np.flatnonzero(np.any(blk != 0.0, axis=0))
        lo, hi = int(nz[0]), int(nz[-1]) + 1
        wins.append((lo, hi, off))
        off += hi - lo
    return wins, off


def _pack(mat_t, wins):
    total = wins[-1][2] + (wins[-1][1] - wins[-1][0])
    p = np.zeros((TILE, total), np.float32)
    for (lo, hi, off), t0 in zip(wins, range(0, mat_t.shape[0], TILE)):
        blk = mat_t[t0:t0 + TILE, lo:hi]
        p[:blk.shape[0], off:off + (hi - lo)] = blk
    return p


def _oh_chunks(n):
    return [(a, min(a + TILE, n)) for a in range(0, n, TILE)]


def _build_program(C, H, W, OH, OW, hwins, wwins, W1, W2, repeat=1,
                   pass_wts=(8, 7), out_bf16=True, xbufs=3, o1bufs=16,
                   osbufs=9):
    from concourse import bass, tile, mybir

    f32 = mybir.dt.float32
    cdt = mybir.dt.bfloat16
    odt = cdt if out_bf16 else f32
    nc = bass.Bass()
    x_d = nc.declare_dram_parameter("x", [C, H, W], cdt, isOutput=False)
    wht_d = nc.declare_dram_parameter("wht", [TILE, W1], cdt, isOutput=False)
    wwt_d = nc.declare_dram_parameter("wwt", [TILE, W2], cdt, isOutput=False)
    out_d = nc.declare_dram_parameter("out", [C, OH, OW], odt, isOutput=True)

    HT = (H + TILE - 1) // TILE
    WT = (W + TILE - 1) // TILE
    tf = H // TILE
    ohc = _oh_chunks(OH)
    groups = []
    wt0 = 0
    for n in pass_wts:
        groups.append(list(range(wt0, min(wt0 + n, WT))))
        wt0 += n
    assert wt0 >= WT and sum(len(g) for g in groups) == WT

    def load_channel(nc, xc, c, granular):
        """All loads on the sync HWDGE queue (strict FIFO = global stream
        order; keeps triggers off the ACT engine). granular: per-h-tile for
        the first channel; 2-tile chunks otherwise."""
        if granular:
            for ht in range(HT):
                p = min(TILE, H - TILE * ht)
                # head tiles trigger from otherwise-idle engines so the
                # stream head drains three queues in parallel; tile0 on
                # scalar (HWDGE ~0.65us to first byte vs SWDGE ~2.6us)
                eng = {0: nc.scalar, 1: nc.gpsimd}.get(ht, nc.sync)
                eng.dma_start(
                    out=xc[0:p, ht * W:ht * W + W],
                    in_=x_d[c, TILE * ht:TILE * ht + p, :],
                )
        else:
            for t0 in range(0, tf, 2):
                nt = min(2, tf - t0)
                nc.sync.dma_start(
                    out=xc[0:TILE, t0 * W:(t0 + nt) * W].rearrange(
                        "p (t w) -> p t w", t=nt),
                    in_=x_d[c, t0 * TILE:(t0 + nt) * TILE, :].rearrange(
                        "(t p) w -> p t w", p=TILE),
                )
            pr = H - tf * TILE
            if pr:
                nc.sync.dma_start(
                    out=xc[0:pr, tf * W:tf * W + W], in_=x_d[c, tf * TILE:H, :])

    with tile.TileContext(nc) as tc:
        with (
            tc.tile_pool(name="consts", bufs=1) as cpool,
            tc.tile_pool(name="xch", bufs=xbufs) as xpool,
            tc.tile_pool(name="o1", bufs=o1bufs) as o1pool,
            tc.tile_pool(name="osb", bufs=osbufs) as opool,
            tc.tile_pool(name="ps", bufs=8, space=bass.MemorySpace.PSUM) as pspool,
        ):
            wht_sb = cpool.tile([TILE, W1], cdt)
            nc.sync.dma_start(out=wht_sb[:, :], in_=wht_d[:, :])
            wwt_sb = cpool.tile([TILE, W2], cdt)
            nc.sync.dma_start(out=wwt_sb[:, :], in_=wwt_d[:, :])

            # upfront loads for the first xbufs bodies, in channel order
            xcs = {}
            nup = min(xbufs, repeat * C)
            for rc in range(nup):
                xc = xpool.tile([TILE, HT * W], cdt, name="xc")
                load_channel(nc, xc, rc % C, granular=(rc == 0))
                xcs[rc] = xc

            for rc in range(repeat * C):
                c = rc % C
                if rc in xcs:
                    xc = xcs.pop(rc)
                else:
                    xc = xpool.tile([TILE, HT * W], cdt, name="xc")
                    load_channel(nc, xc, c, granular=False)

                o1s = [None] * WT
                ps2s = None

                def emit_s2(grp):
                    for wt in grp:
                        o1, pw = o1s[wt]
                        wlo, whi, woff = wwins[wt]
                        for k, (a, b) in enumerate(ohc):
                            nc.tensor.matmul(
                                ps2s[k][0:b - a, wlo:whi],
                                o1[0:pw, a:b],
                                wwt_sb[0:pw, woff:woff + (whi - wlo)],
                                start=(wt == 0),
                                stop=(wt == WT - 1),
                            )

                # Two full-H passes of (8, 7) w-tiles. (H-splitting the
                # passes to recycle PSUM banks mid-load was measured: it
                # speeds the first channel ~1.4us but the gain doesn't
                # propagate — later channels are gated by their own stream
                # positions — and applying it to all channels costs more in
                # merge-eviction overhead than it saves: 61.7/63.6us vs
                # 61.1-61.6us for this plain config.)
                phases = [(g, 0, HT) for g in groups]
                split_lo, split_hi = hwins[5][0], hwins[4][1]
                for gi, (grp, h0, h1) in enumerate(phases):
                    ps1s = {}
                    for wt in grp:
                        ps1s[wt] = pspool.tile([TILE, OH], f32, name="ps1",
                                               tag="ps")
                    for ht in range(h0, h1):
                        p = min(TILE, H - TILE * ht)
                        lo, hi, off = hwins[ht]
                        for wt in grp:
                            pw = min(TILE, W - TILE * wt)
                            nc.tensor.matmul(
                                ps1s[wt][0:pw, lo:hi],
                                xc[0:p, ht * W + TILE * wt:ht * W + TILE * wt + pw],
                                wht_sb[0:p, off:off + (hi - lo)],
                                start=(ht == h0),
                                stop=(ht == h1 - 1),
                            )
                    # evict this pass's accumulators
                    for j, wt in enumerate(grp):
                        pw = min(TILE, W - TILE * wt)
                        if h0 == 0 and h1 == HT:
                            o1 = o1pool.tile([TILE, OH], cdt, name="o1")
                            if j % 2 == 0:
                                nc.vector.tensor_copy(o1[0:pw, :],
                                                      ps1s[wt][0:pw, :])
                            else:
                                nc.scalar.copy(o1[0:pw, :], ps1s[wt][0:pw, :])
                            o1s[wt] = (o1, pw)
                        elif h0 == 0:
                            # H-split first half: partial in f32; the psum is
                            # only valid in [0, split_hi) (union of ht0..4
                            # windows)
                            o1a = o1pool.tile([TILE, OH], f32, name="o1a",
                                              tag="o1a")
                            if j % 2 == 0:
                                nc.vector.tensor_copy(
                                    o1a[0:pw, 0:split_hi],
                                    ps1s[wt][0:pw, 0:split_hi])
                            else:
                                nc.scalar.copy(o1a[0:pw, 0:split_hi],
                                               ps1s[wt][0:pw, 0:split_hi])
                            o1s[wt] = (o1a, pw)
                        else:
                            # second half covers [split_lo, OH): merge the two
                            # valid ranges (overlap [split_lo, split_hi) adds)
                            o1a, pw = o1s[wt]
                            o1 = o1pool.tile([TILE, OH], cdt, name="o1")
                            nc.scalar.copy(o1[0:pw, 0:split_lo],
                                           o1a[0:pw, 0:split_lo])
                            nc.vector.tensor_add(
                                o1[0:pw, split_lo:split_hi],
                                ps1s[wt][0:pw, split_lo:split_hi],
                                o1a[0:pw, split_lo:split_hi])
                            nc.vector.tensor_copy(o1[0:pw, split_hi:OH],
                                                  ps1s[wt][0:pw, split_hi:OH])
                            o1s[wt] = (o1, pw)
                    if gi == len(phases) - 1:
                        ps2s = [pspool.tile([TILE, OW], f32, name="ps2",
                                            tag="ps") for _ in ohc]
                for grp in groups:
                    emit_s2(grp)

                for k, (a, b) in enumerate(ohc):
                    osb = opool.tile([TILE, OW], odt, name="osb")
                    nc.scalar.copy(osb[0:b - a, :], ps2s[k][0:b - a, :])
                    nc.sync.dma_start(out=out_d[c, a:b, :], in_=osb[0:b - a, :])

    _split_multi_waits(nc)
    return nc


def _as_bf16(a):
    import ml_dtypes
    return np.asarray(a, np.float32).astype(ml_dtypes.bfloat16)


def kernel(x, w_h, idx_h, w_w, idx_w, _trace=False):
    _ensure_concourse()
    _patch_tile_drain()
    from concourse.bass_utils import run_bass_kernel_spmd

    x = np.ascontiguousarray(np.asarray(x, np.float32))
    B, C, H, W = x.shape
    wht_t = _dense_t(w_h, idx_h, H)   # [H, OH]
    wwt_t = _dense_t(w_w, idx_w, W)   # [W, OW]
    OH, OW = wht_t.shape[1], wwt_t.shape[1]

    hwins, W1 = _windows(wht_t)
    wwins, W2 = _windows(wwt_t)
    wht_packed = _as_bf16(_pack(wht_t, hwins))
    wwt_packed = _as_bf16(_pack(wwt_t, wwins))
    x_bf16 = _as_bf16(x)

    nc = _build_program(C, H, W, OH, OW, hwins, wwins, W1, W2)

    in_maps = [
        {"x": x_bf16[b], "wht": wht_packed, "wwt": wwt_packed} for b in range(B)
    ]
    res = run_bass_kernel_spmd(nc, in_maps, list(range(B)), trace=bool(_trace))
    out = np.stack(
        [np.asarray(res.results[i]["out"], np.float32) for i in range(B)],
        axis=0,
    )
    if _trace:
        return out, res
    return out


# revision 21
# speedup vs baseline: 1.0006x; 1.0006x over previous
"""Bicubic 4x downsample (MATLAB imresize-style) on Trainium2, 8-core data parallel.

Math: separable resize, H then W; both stages are banded matmuls evaluated
on the PE array with the image tile as the stationary operand:

  stage 1:  o1[w, oh]  = sum_h  X[h, w]  * WHT[h, oh]    (per 128x128 x-tile,
            streaming only the ~35-wide band window of WHT per h-tile)
  stage 2:  out[oh,ow] = sum_w  o1[w,oh] * WWT[w, ow]

Perf structure (single-shot exec time is the metric):
- x is cast to bf16 on the HOST: halves HBM read traffic (the kernel is
  HBM-stream-bound), and bf16 operands run the PE at 1 cycle/row.
- The output is stored bf16 (upcast on host); rel-err stays ~5e-3.
- All load DMAs ride ONE HWDGE queue (sync) in strict channel order; DMA
  trigger instructions are kept off the scalar/ACT engine, which runs the
  PSUM evictions (triggers are ~0.7us each and would delay evictions).
- s1 runs ht-major in two passes of (8, 7) w-tiles: channel 0's pass0
  tracks the load wave h-tile by h-tile instead of needing the whole
  channel resident; 8 PSUM banks are shared by the s1 accumulators and
  the 3 s2 accumulators via one rotating pool.
- o1 evictions alternate DVE/ACT so ps1 bank reuse never waits on one
  engine; output stores trigger on sync (idle after the load triggers).

Sharding: pure data parallel, batch b -> core b (8 batches, 8 cores).
"""

import numpy as np

TILE = 128


def _ensure_concourse():
    try:
        import concourse  # noqa: F401
    except ImportError:
        import sys
        for p in ("/opt/trn_rl_repo", "/root/.axon_site/_ro/trn_rl_repo"):
            if p not in sys.path:
                sys.path.insert(0, p)


_PATCHED = False


def _patch_tile_drain():
    """This walrus build rejects >1 sem wait on TPB_CTRL instructions (the
    Tile exit Drain). Split the final drain's waits into single-wait nops."""
    global _PATCHED
    if _PATCHED:
        return
    from concourse import tile
    from concourse.vector_clock import VectorClock, ScopedClock

    def _drain_and_barrier(self, tick_clock, wait_clock):
        gc = tick_clock.global_clock
        n = len(gc)
        for i in range(n):
            if gc[i] <= 0:
                continue
            vc = VectorClock([gc[j] if j == i else 0 for j in range(n)])
            nop_inst = self.nc.sync.nop(nofuse=True, hint="drain_split")
            wait_clock.add_sem_waits(nop_inst.ins, ScopedClock({None: vc}))
        self.nc.sync.drain()
        self.nc.all_engine_barrier()
        assert self.sems is not None
        popped = self.nc._tile_sem_poison_stack.pop()
        assert popped is self._sem_poison
        self.nc.clear_and_free_semaphores(list(self.sems.allocated().values()))
        self.nc.all_engine_barrier()

    tile.TileContext._drain_and_barrier = _drain_and_barrier
    _PATCHED = True


def _split_multi_waits(nc):
    """This walrus build rejects instructions carrying >1 sem wait. Hoist all
    but the last wait of any instruction onto same-engine nops placed
    immediately before it (engine streams execute block order in-order, so
    waiting on a preceding nop is equivalent)."""
    from concourse import mybir

    uid = 0
    for fn in nc.m.functions:
        for bb in fn.blocks:
            insts = bb.instructions  # live list
            new_list = []
            changed = False
            for ins in list(insts):
                si = ins.sync_info
                if si is not None and len(si.on_wait) > 1:
                    waits = list(si.on_wait)
                    for wt in waits[:-1]:
                        uid += 1
                        nop = mybir.InstNoOp(
                            name=f"ws_nop_{uid}",
                            engine=ins.engine,
                            ins=[],
                            outs=[],
                            sync_info=mybir.SyncInfo(on_wait=[wt], on_update=[]),
                            bass_nofuse=True,
                        )
                        new_list.append(nop)
                    ins.sync_info = mybir.SyncInfo(
                        on_wait=[waits[-1]], on_update=list(si.on_update)
                    )
                    changed = True
                new_list.append(ins)
            if changed:
                insts.clear()
                insts.extend(new_list)


def _dense_t(weights, indices, in_len):
    """Dense transposed resize matrix [in_len, out_len]:
    M[i, o] = sum over taps p with indices[o, p] == i of weights[o, p]."""
    w = np.asarray(weights, np.float32)
    idx = np.asarray(indices, np.int64)
    out_len, ntap = w.shape
    m = np.zeros((in_len, out_len), np.float32)
    ocol = np.repeat(np.arange(out_len), ntap)
    np.add.at(m, (idx.ravel(), ocol), w.ravel())
    return m


def _windows(mat_t):
    """Per 128-row tile of the [in, out] matrix: (out_lo, out_hi, packed_off)."""
    wins = []
    off = 0
    for t0 in range(0, mat_t.shape[0], TILE):
        blk = mat_t[t0:t0 + TILE]
        nz = np.flatnonzero(np.any(blk != 0.0, axis=0))
        lo, hi = int(nz[0]), int(nz[-1]) + 1
        wins.append((lo, hi, off))
        off += hi - lo
    return wins, off


def _pack(mat_t, wins):
    total = wins[-1][2] + (wins[-1][1] - wins[-1][0])
    p = np.zeros((TILE, total), np.float32)
    for (lo, hi, off), t0 in zip(wins, range(0, mat_t.shape[0], TILE)):
        blk = mat_t[t0:t0 + TILE, lo:hi]
        p[:blk.shape[0], off:off + (hi - lo)] = blk
    return p


def _oh_chunks(n):
    return [(a, min(a + TILE, n)) for a in range(0, n, TILE)]


def _build_program(C, H, W, OH, OW, hwins, wwins, W1, W2, repeat=1,
                   pass_wts=(8, 7), out_bf16=True, xbufs=3, o1bufs=16,
                   osbufs=4):
    from concourse import bass, tile, mybir

    f32 = mybir.dt.float32
    cdt = mybir.dt.bfloat16
    odt = cdt if out_bf16 else f32
    nc = bass.Bass()
    x_d = nc.declare_dram_parameter("x", [C, H, W], cdt, isOutput=False)
    wht_d = nc.declare_dram_parameter("wht", [TILE, W1], cdt, isOutput=False)
    wwt_d = nc.declare_dram_parameter("wwt", [TILE, W2], cdt, isOutput=False)
    out_d = nc.declare_dram_parameter("out", [C, OH, OW], odt, isOutput=True)

    HT = (H + TILE - 1) // TILE
    WT = (W + TILE - 1) // TILE
    tf = H // TILE
    ohc = _oh_chunks(OH)
    groups = []
    wt0 = 0
    for n in pass_wts:
        groups.append(list(range(wt0, min(wt0 + n, WT))))
        wt0 += n
    assert wt0 >= WT and sum(len(g) for g in groups) == WT

    def load_channel(nc, xc, c, granular):
        """All loads on the sync HWDGE queue (strict FIFO = global stream
        order; keeps triggers off the ACT engine). granular: per-h-tile for
        the first channel; 2-tile chunks otherwise."""
        if granular:
            for ht in range(HT):
                p = min(TILE, H - TILE * ht)
                # head tiles trigger from otherwise-idle engines so the
                # stream head drains three queues in parallel; tile0 on
                # scalar (HWDGE ~0.65us to first byte vs SWDGE ~2.6us)
                eng = {0: nc.scalar, 1: nc.gpsimd}.get(ht, nc.sync)
                eng.dma_start(
                    out=xc[0:p, ht * W:ht * W + W],
                    in_=x_d[c, TILE * ht:TILE * ht + p, :],
                )
        else:
            for t0 in range(0, tf, 2):
                nt = min(2, tf - t0)
                nc.sync.dma_start(
                    out=xc[0:TILE, t0 * W:(t0 + nt) * W].rearrange(
                        "p (t w) -> p t w", t=nt),
                    in_=x_d[c, t0 * TILE:(t0 + nt) * TILE, :].rearrange(
                        "(t p) w -> p t w", p=TILE),
                )
            pr = H - tf * TILE
            if pr:
                nc.sync.dma_start(
                    out=xc[0:pr, tf * W:tf * W + W], in_=x_d[c, tf * TILE:H, :])

    with tile.TileContext(nc) as tc:
        with (
            tc.tile_pool(name="consts", bufs=1) as cpool,
            tc.tile_pool(name="xch", bufs=xbufs) as xpool,
            tc.tile_pool(name="o1", bufs=o1bufs) as o1pool,
            tc.tile_pool(name="osb", bufs=osbufs) as opool,
            tc.tile_pool(name="ps", bufs=8, space=bass.MemorySpace.PSUM) as pspool,
        ):
            wht_sb = cpool.tile([TILE, W1], cdt)
            nc.sync.dma_start(out=wht_sb[:, :], in_=wht_d[:, :])
            wwt_sb = cpool.tile([TILE, W2], cdt)
            nc.sync.dma_start(out=wwt_sb[:, :], in_=wwt_d[:, :])

            # upfront loads for the first xbufs bodies, in channel order
            xcs = {}
            nup = min(xbufs, repeat * C)
            for rc in range(nup):
                xc = xpool.tile([TILE, HT * W], cdt, name="xc")
                load_channel(nc, xc, rc % C, granular=(rc == 0))
                xcs[rc] = xc

            for rc in range(repeat * C):
                c = rc % C
                if rc in xcs:
                    xc = xcs.pop(rc)
                else:
                    xc = xpool.tile([TILE, HT * W], cdt, name="xc")
                    load_channel(nc, xc, c, granular=False)

                o1s = [None] * WT
                ps2s = None

                def emit_s2(grp):
                    for wt in grp:
                        o1, pw = o1s[wt]
                        wlo, whi, woff = wwins[wt]
                        for k, (a, b) in enumerate(ohc):
                            nc.tensor.matmul(
                                ps2s[k][0:b - a, wlo:whi],
                                o1[0:pw, a:b],
                                wwt_sb[0:pw, woff:woff + (whi - wlo)],
                                start=(wt == 0),
                                stop=(wt == WT - 1),
                            )

                # Two full-H passes of (8, 7) w-tiles. (H-splitting the
                # passes to recycle PSUM banks mid-load was measured: it
                # speeds the first channel ~1.4us but the gain doesn't
                # propagate — later channels are gated by their own stream
                # positions — and applying it to all channels costs more in
                # merge-eviction overhead than it saves: 61.7/63.6us vs
                # 61.1-61.6us for this plain config.)
                phases = [(g, 0, HT) for g in groups]
                split_lo, split_hi = hwins[5][0], hwins[4][1]
                for gi, (grp, h0, h1) in enumerate(phases):
                    ps1s = {}
                    for wt in grp:
                        ps1s[wt] = pspool.tile([TILE, OH], f32, name="ps1",
                                               tag="ps")
                    for ht in range(h0, h1):
                        p = min(TILE, H - TILE * ht)
                        lo, hi, off = hwins[ht]
                        for wt in grp:
                            pw = min(TILE, W - TILE * wt)
                            nc.tensor.matmul(
                                ps1s[wt][0:pw, lo:hi],
                                xc[0:p, ht * W + TILE * wt:ht * W + TILE * wt + pw],
                                wht_sb[0:p, off:off + (hi - lo)],
                                start=(ht == h0),
                                stop=(ht == h1 - 1),
                            )
                    # evict this pass's accumulators
                    for j, wt in enumerate(grp):
                        pw = min(TILE, W - TILE * wt)
                        if h0 == 0 and h1 == HT:
                            o1 = o1pool.tile([TILE, OH], cdt, name="o1")
                            if j % 2 == 0:
                                nc.vector.tensor_copy(o1[0:pw, :],
                                                      ps1s[wt][0:pw, :])
                            else:
                                nc.scalar.copy(o1[0:pw, :], ps1s[wt][0:pw, :])
                            o1s[wt] = (o1, pw)
                        elif h0 == 0:
                            # H-split first half: partial in f32; the psum is
                            # only valid in [0, split_hi) (union of ht0..4
                            # windows)
                            o1a = o1pool.tile([TILE, OH], f32, name="o1a",
                                              tag="o1a")
                            if j % 2 == 0:
                                nc.vector.tensor_copy(
                                    o1a[0:pw, 0:split_hi],
                                    ps1s[wt][0:pw, 0:split_hi])
                            else:
                                nc.scalar.copy(o1a[0:pw, 0:split_hi],
                                               ps1s[wt][0:pw, 0:split_hi])
                            o1s[wt] = (o1a, pw)
                        else:
                            # second half covers [split_lo, OH): merge the two
                            # valid ranges (overlap [split_lo, split_hi) adds)
                            o1a, pw = o1s[wt]
                            o1 = o1pool.tile([TILE, OH], cdt, name="o1")
                            nc.scalar.copy(o1[0:pw, 0:split_lo],
                                           o1a[0:pw, 0:split_lo])
                            nc.vector.tensor_add(
                                o1[0:pw, split_lo:split_hi],
                                ps1s[wt][0:pw, split_lo:split_hi],
                                o1a[0:pw, split_lo:split_hi])
                            nc.vector.tensor_copy(o1[0:pw, split_hi:OH],
                                                  ps1s[wt][0:pw, split_hi:OH])
                            o1s[wt] = (o1, pw)
                    if gi == len(phases) - 1:
                        ps2s = [pspool.tile([TILE, OW], f32, name="ps2",
                                            tag="ps") for _ in ohc]
                for grp in groups:
                    emit_s2(grp)

                for k, (a, b) in enumerate(ohc):
                    osb = opool.tile([TILE, OW], odt, name="osb")
                    # split the three output evicts across DVE/ACT so the
                    # serial evict chain in the channel tail halves
                    if k == 1:
                        nc.scalar.copy(osb[0:b - a, :], ps2s[k][0:b - a, :])
                    else:
                        nc.vector.tensor_copy(osb[0:b - a, :],
                                              ps2s[k][0:b - a, :])
                    nc.sync.dma_start(out=out_d[c, a:b, :], in_=osb[0:b - a, :])

    _split_multi_waits(nc)
    return nc


def _as_bf16(a):
    import ml_dtypes
    return np.asarray(a, np.float32).astype(ml_dtypes.bfloat16)


def kernel(x, w_h, idx_h, w_w, idx_w, _trace=False):
    _ensure_concourse()
    _patch_tile_drain()
    from concourse.bass_utils import run_bass_kernel_spmd

    x = np.ascontiguousarray(np.asarray(x, np.float32))
    B, C, H, W = x.shape
    wht_t = _dense_t(w_h, idx_h, H)   # [H, OH]
    wwt_t = _dense_t(w_w, idx_w, W)   # [W, OW]
    OH, OW = wht_t.shape[1], wwt_t.shape[1]

    hwins, W1 = _windows(wht_t)
    wwins, W2 = _windows(wwt_t)
    wht_packed = _as_bf16(_pack(wht_t, hwins))
    wwt_packed = _as_bf16(_pack(wwt_t, wwins))
    x_bf16 = _as_bf16(x)

    nc = _build_program(C, H, W, OH, OW, hwins, wwins, W1, W2)

    in_maps = [
        {"x": x_bf16[b], "wht": wht_packed, "wwt": wwt_packed} for b in range(B)
    ]
    res = run_bass_kernel_spmd(nc, in_maps, list(range(B)), trace=bool(_trace))
    out = np.stack(
        [np.asarray(res.results[i]["out"], np.float32) for i in range(B)],
        axis=0,
    )
    if _trace:
        return out, res
    return out


# revision 22
# speedup vs baseline: 1.0404x; 1.0398x over previous
"""Bicubic 4x downsample (MATLAB imresize-style) on Trainium2, 8-core data parallel.

Math: separable resize, H then W; both stages are banded matmuls evaluated
on the PE array with the image tile as the stationary operand:

  stage 1:  o1[w, oh]  = sum_h  X[h, w]  * WHT[h, oh]    (per 128x128 x-tile,
            streaming only the ~35-wide band window of WHT per h-tile)
  stage 2:  out[oh,ow] = sum_w  o1[w,oh] * WWT[w, ow]

Perf structure (single-shot exec time is the metric):
- x is cast to bf16 on the HOST: halves HBM read traffic (the kernel is
  HBM-stream-bound), and bf16 operands run the PE at 1 cycle/row.
- The output is stored bf16 (upcast on host); rel-err stays ~5e-3.
- All load DMAs ride ONE HWDGE queue (sync) in strict channel order; DMA
  trigger instructions are kept off the scalar/ACT engine, which runs the
  PSUM evictions (triggers are ~0.7us each and would delay evictions).
- s1 runs ht-major in two passes of (8, 7) w-tiles: channel 0's pass0
  tracks the load wave h-tile by h-tile instead of needing the whole
  channel resident; 8 PSUM banks are shared by the s1 accumulators and
  the 3 s2 accumulators via one rotating pool.
- o1 evictions alternate DVE/ACT so ps1 bank reuse never waits on one
  engine; output stores trigger on sync (idle after the load triggers).

Sharding: pure data parallel, batch b -> core b (8 batches, 8 cores).
"""

import numpy as np

TILE = 128


def _ensure_concourse():
    try:
        import concourse  # noqa: F401
    except ImportError:
        import sys
        for p in ("/opt/trn_rl_repo", "/root/.axon_site/_ro/trn_rl_repo"):
            if p not in sys.path:
                sys.path.insert(0, p)


_PATCHED = False


def _patch_tile_drain():
    """This walrus build rejects >1 sem wait on TPB_CTRL instructions (the
    Tile exit Drain). Split the final drain's waits into single-wait nops."""
    global _PATCHED
    if _PATCHED:
        return
    from concourse import tile
    from concourse.vector_clock import VectorClock, ScopedClock

    def _drain_and_barrier(self, tick_clock, wait_clock):
        gc = tick_clock.global_clock
        n = len(gc)
        for i in range(n):
            if gc[i] <= 0:
                continue
            vc = VectorClock([gc[j] if j == i else 0 for j in range(n)])
            nop_inst = self.nc.sync.nop(nofuse=True, hint="drain_split")
            wait_clock.add_sem_waits(nop_inst.ins, ScopedClock({None: vc}))
        self.nc.sync.drain()
        self.nc.all_engine_barrier()
        assert self.sems is not None
        popped = self.nc._tile_sem_poison_stack.pop()
        assert popped is self._sem_poison
        self.nc.clear_and_free_semaphores(list(self.sems.allocated().values()))
        self.nc.all_engine_barrier()

    tile.TileContext._drain_and_barrier = _drain_and_barrier
    _PATCHED = True


def _split_multi_waits(nc):
    """This walrus build rejects instructions carrying >1 sem wait. Hoist all
    but the last wait of any instruction onto same-engine nops placed
    immediately before it (engine streams execute block order in-order, so
    waiting on a preceding nop is equivalent)."""
    from concourse import mybir

    uid = 0
    for fn in nc.m.functions:
        for bb in fn.blocks:
            insts = bb.instructions  # live list
            new_list = []
            changed = False
            for ins in list(insts):
                si = ins.sync_info
                if si is not None and len(si.on_wait) > 1:
                    waits = list(si.on_wait)
                    for wt in waits[:-1]:
                        uid += 1
                        nop = mybir.InstNoOp(
                            name=f"ws_nop_{uid}",
                            engine=ins.engine,
                            ins=[],
                            outs=[],
                            sync_info=mybir.SyncInfo(on_wait=[wt], on_update=[]),
                            bass_nofuse=True,
                        )
                        new_list.append(nop)
                    ins.sync_info = mybir.SyncInfo(
                        on_wait=[waits[-1]], on_update=list(si.on_update)
                    )
                    changed = True
                new_list.append(ins)
            if changed:
                insts.clear()
                insts.extend(new_list)


def _dense_t(weights, indices, in_len):
    """Dense transposed resize matrix [in_len, out_len]:
    M[i, o] = sum over taps p with indices[o, p] == i of weights[o, p]."""
    w = np.asarray(weights, np.float32)
    idx = np.asarray(indices, np.int64)
    out_len, ntap = w.shape
    m = np.zeros((in_len, out_len), np.float32)
    ocol = np.repeat(np.arange(out_len), ntap)
    np.add.at(m, (idx.ravel(), ocol), w.ravel())
    return m


def _windows(mat_t):
    """Per 128-row tile of the [in, out] matrix: (out_lo, out_hi, packed_off)."""
    wins = []
    off = 0
    for t0 in range(0, mat_t.shape[0], TILE):
        blk = mat_t[t0:t0 + TILE]
        nz = np.flatnonzero(np.any(blk != 0.0, axis=0))
        lo, hi = int(nz[0]), int(nz[-1]) + 1
        wins.append((lo, hi, off))
        off += hi - lo
    return wins, off


def _pack(mat_t, wins):
    total = wins[-1][2] + (wins[-1][1] - wins[-1][0])
    p = np.zeros((TILE, total), np.float32)
    for (lo, hi, off), t0 in zip(wins, range(0, mat_t.shape[0], TILE)):
        blk = mat_t[t0:t0 + TILE, lo:hi]
        p[:blk.shape[0], off:off + (hi - lo)] = blk
    return p


def _oh_chunks(n):
    return [(a, min(a + TILE, n)) for a in range(0, n, TILE)]


def _build_program(C, H, W, OH, OW, hwins, wwins, W1, W2, repeat=1,
                   pass_wts=(8, 7), out_bf16=True, xbufs=3, o1bufs=16,
                   osbufs=4):
    from concourse import bass, tile, mybir

    f32 = mybir.dt.float32
    cdt = mybir.dt.bfloat16
    odt = cdt if out_bf16 else f32
    nc = bass.Bass()
    x_d = nc.declare_dram_parameter("x", [C, H, W], cdt, isOutput=False)
    wht_d = nc.declare_dram_parameter("wht", [TILE, W1], cdt, isOutput=False)
    wwt_d = nc.declare_dram_parameter("wwt", [TILE, W2], cdt, isOutput=False)
    out_d = nc.declare_dram_parameter("out", [C, OH, OW], odt, isOutput=True)

    HT = (H + TILE - 1) // TILE
    WT = (W + TILE - 1) // TILE
    tf = H // TILE
    ohc = _oh_chunks(OH)
    groups = []
    wt0 = 0
    for n in pass_wts:
        groups.append(list(range(wt0, min(wt0 + n, WT))))
        wt0 += n
    assert wt0 >= WT and sum(len(g) for g in groups) == WT

    def load_channel(nc, xc, c, granular):
        """All loads on the sync HWDGE queue (strict FIFO = global stream
        order; keeps triggers off the ACT engine). granular: per-h-tile for
        the first channel; 2-tile chunks otherwise."""
        if granular:
            for ht in range(HT):
                p = min(TILE, H - TILE * ht)
                # head tiles trigger from otherwise-idle engines so the
                # stream head drains three queues in parallel; tile0 on
                # scalar (HWDGE ~0.65us to first byte vs SWDGE ~2.6us)
                eng = {0: nc.scalar, 1: nc.gpsimd}.get(ht, nc.sync)
                eng.dma_start(
                    out=xc[0:p, ht * W:ht * W + W],
                    in_=x_d[c, TILE * ht:TILE * ht + p, :],
                )
        else:
            for t0 in range(0, tf, 2):
                nt = min(2, tf - t0)
                nc.sync.dma_start(
                    out=xc[0:TILE, t0 * W:(t0 + nt) * W].rearrange(
                        "p (t w) -> p t w", t=nt),
                    in_=x_d[c, t0 * TILE:(t0 + nt) * TILE, :].rearrange(
                        "(t p) w -> p t w", p=TILE),
                )
            pr = H - tf * TILE
            if pr:
                nc.sync.dma_start(
                    out=xc[0:pr, tf * W:tf * W + W], in_=x_d[c, tf * TILE:H, :])

    with tile.TileContext(nc) as tc:
        with (
            tc.tile_pool(name="consts", bufs=1) as cpool,
            tc.tile_pool(name="xch", bufs=xbufs) as xpool,
            tc.tile_pool(name="o1", bufs=o1bufs) as o1pool,
            tc.tile_pool(name="osb", bufs=osbufs) as opool,
            tc.tile_pool(name="ps", bufs=8, space=bass.MemorySpace.PSUM) as pspool,
        ):
            wht_sb = cpool.tile([TILE, W1], cdt)
            nc.sync.dma_start(out=wht_sb[:, :], in_=wht_d[:, :])
            wwt_sb = cpool.tile([TILE, W2], cdt)
            nc.sync.dma_start(out=wwt_sb[:, :], in_=wwt_d[:, :])

            # upfront loads for the first xbufs bodies, in channel order
            xcs = {}
            nup = min(xbufs, repeat * C)
            for rc in range(nup):
                xc = xpool.tile([TILE, HT * W], cdt, name="xc")
                load_channel(nc, xc, rc % C, granular=(rc == 0))
                xcs[rc] = xc

            for rc in range(repeat * C):
                c = rc % C
                if rc in xcs:
                    xc = xcs.pop(rc)
                else:
                    xc = xpool.tile([TILE, HT * W], cdt, name="xc")
                    load_channel(nc, xc, c, granular=False)

                o1s = [None] * WT
                ps2s = None

                def emit_s2(grp):
                    for wt in grp:
                        o1, pw = o1s[wt]
                        wlo, whi, woff = wwins[wt]
                        for k, (a, b) in enumerate(ohc):
                            nc.tensor.matmul(
                                ps2s[k][0:b - a, wlo:whi],
                                o1[0:pw, a:b],
                                wwt_sb[0:pw, woff:woff + (whi - wlo)],
                                start=(wt == 0),
                                stop=(wt == WT - 1),
                            )

                # Two full-H passes of (8, 7) w-tiles. (H-splitting the
                # passes to recycle PSUM banks mid-load was measured: it
                # speeds the first channel ~1.4us but the gain doesn't
                # propagate — later channels are gated by their own stream
                # positions — and applying it to all channels costs more in
                # merge-eviction overhead than it saves: 61.7/63.6us vs
                # 61.1-61.6us for this plain config.)
                phases = [(g, 0, HT) for g in groups]
                split_lo, split_hi = hwins[5][0], hwins[4][1]
                for gi, (grp, h0, h1) in enumerate(phases):
                    ps1s = {}
                    for wt in grp:
                        ps1s[wt] = pspool.tile([TILE, OH], f32, name="ps1",
                                               tag="ps")
                    for ht in range(h0, h1):
                        p = min(TILE, H - TILE * ht)
                        lo, hi, off = hwins[ht]
                        for wt in grp:
                            pw = min(TILE, W - TILE * wt)
                            nc.tensor.matmul(
                                ps1s[wt][0:pw, lo:hi],
                                xc[0:p, ht * W + TILE * wt:ht * W + TILE * wt + pw],
                                wht_sb[0:p, off:off + (hi - lo)],
                                start=(ht == h0),
                                stop=(ht == h1 - 1),
                            )
                    # evict this pass's accumulators
                    for j, wt in enumerate(grp):
                        pw = min(TILE, W - TILE * wt)
                        if h0 == 0 and h1 == HT:
                            o1 = o1pool.tile([TILE, OH], cdt, name="o1")
                            if j % 2 == 0:
                                nc.vector.tensor_copy(o1[0:pw, :],
                                                      ps1s[wt][0:pw, :])
                            else:
                                nc.scalar.copy(o1[0:pw, :], ps1s[wt][0:pw, :])
                            o1s[wt] = (o1, pw)
                        elif h0 == 0:
                            # H-split first half: partial in f32; the psum is
                            # only valid in [0, split_hi) (union of ht0..4
                            # windows)
                            o1a = o1pool.tile([TILE, OH], f32, name="o1a",
                                              tag="o1a")
                            if j % 2 == 0:
                                nc.vector.tensor_copy(
                                    o1a[0:pw, 0:split_hi],
                                    ps1s[wt][0:pw, 0:split_hi])
                            else:
                                nc.scalar.copy(o1a[0:pw, 0:split_hi],
                                               ps1s[wt][0:pw, 0:split_hi])
                            o1s[wt] = (o1a, pw)
                        else:
                            # second half covers [split_lo, OH): merge the two
                            # valid ranges (overlap [split_lo, split_hi) adds)
                            o1a, pw = o1s[wt]
                            o1 = o1pool.tile([TILE, OH], cdt, name="o1")
                            nc.scalar.copy(o1[0:pw, 0:split_lo],
                                           o1a[0:pw, 0:split_lo])
                            nc.vector.tensor_add(
                                o1[0:pw, split_lo:split_hi],
                                ps1s[wt][0:pw, split_lo:split_hi],
                                o1a[0:pw, split_lo:split_hi])
                            nc.vector.tensor_copy(o1[0:pw, split_hi:OH],
                                                  ps1s[wt][0:pw, split_hi:OH])
                            o1s[wt] = (o1, pw)
                    if gi == len(phases) - 1:
                        ps2s = [pspool.tile([TILE, OW], f32, name="ps2",
                                            tag="ps") for _ in ohc]
                for grp in groups:
                    emit_s2(grp)

                for k, (a, b) in enumerate(ohc):
                    osb = opool.tile([TILE, OW], odt, name="osb")
                    nc.scalar.copy(osb[0:b - a, :], ps2s[k][0:b - a, :])
                    nc.sync.dma_start(out=out_d[c, a:b, :], in_=osb[0:b - a, :])

    _split_multi_waits(nc)
    return nc


def _as_bf16(a):
    import ml_dtypes
    return np.asarray(a, np.float32).astype(ml_dtypes.bfloat16)


def kernel(x, w_h, idx_h, w_w, idx_w, _trace=False):
    _ensure_concourse()
    _patch_tile_drain()
    from concourse.bass_utils import run_bass_kernel_spmd

    x = np.ascontiguousarray(np.asarray(x, np.float32))
    B, C, H, W = x.shape
    wht_t = _dense_t(w_h, idx_h, H)   # [H, OH]
    wwt_t = _dense_t(w_w, idx_w, W)   # [W, OW]
    OH, OW = wht_t.shape[1], wwt_t.shape[1]

    hwins, W1 = _windows(wht_t)
    wwins, W2 = _windows(wwt_t)
    wht_packed = _as_bf16(_pack(wht_t, hwins))
    wwt_packed = _as_bf16(_pack(wwt_t, wwins))
    x_bf16 = _as_bf16(x)

    nc = _build_program(C, H, W, OH, OW, hwins, wwins, W1, W2)

    in_maps = [
        {"x": x_bf16[b], "wht": wht_packed, "wwt": wwt_packed} for b in range(B)
    ]
    res = run_bass_kernel_spmd(nc, in_maps, list(range(B)), trace=bool(_trace))
    out = np.stack(
        [np.asarray(res.results[i]["out"], np.float32) for i in range(B)],
        axis=0,
    )
    if _trace:
        return out, res
    return out
